# revision 28
# baseline (speedup 1.0000x reference)
"""Bahdanau-attention kernel for trn2, data-parallel over batch across 8 cores.

Per-core computation (B_LOC = 4 batches, S = 4096, H = E = 256):
  energy = tanh(hidden @ Wh.T + enc @ We.T + b_attn)      [b, s, e]
  scores = energy . v                                      [b, s]
  attn   = softmax(scores) over s  (no max-subtraction: scores bounded by ||v||_1)
  out    = sum_s attn * enc                                [b, h]

Design (fp8-pair xbar transposes + DoubleRow split-We energy):
  - enc slice read from HBM once via SWDGE cast-DMA (f32 -> bf16) in
    1-MiB i-halves, 4 consecutive s-rows per partition (16KB-contiguous
    read descriptors). The SDMA system moves ~460 GB/s summed over both
    sides of every transfer, so total DMA bytes are the budget that
    matters: this design moves 33.6 MB vs v1's 41.8 MB.
  - The energy matmul needs X^T (h on partitions). DVE casts X to fp8e4
    with the two h-halves INTERLEAVED along the free dim (h' = 2k + hh);
    a bf16-bitcast xbar transpose then moves 16-bit h-PAIRS: partition k
    of the output holds (h=k, h=128+k) interleaved along free — exactly
    the two k-tile planes of a DoubleRow matmul. Transpose bytes drop 4x
    vs transposing bf16 X directly.
  - Energy: fp8 DoubleRow matmuls (full 256-deep contraction per MM, the
    interleaved pairs fetched as contiguous 16-bit reads at 2 rows/cyc).
    fp8 We alone costs 1.7e-2 rel (host-sim) — so We is SPLIT into
    value + residual fp8 terms (both pre-scaled x32, descaled by the
    tanh's scale=1/32), which recovers ~bf16 weight accuracy; fp8 X
    contributes the remaining ~5.5e-3.
  - tanh folds the per-partition bias qb[e] = hidden @ Wh.T + b_attn; the
    v-dot runs on the PE with v stationary, 4 batches' score strips in one
    PSUM bank at partitions {0,32,64,96} (tile_position col packing).
  - One Exp per group produces exp(scores) + denominators (accum_out);
    exp strips are PE-transposed to [s-part, b]; the unnormalized context
    accumulates in ONE shared PSUM bank (halves at row offsets 0/32 via
    tile_position) against the resident native bf16 X (full-precision
    path), freeing a bank for a 5-deep energy-PSUM pool. Softmax division
    happens on the host.
  - DMA surgery: reads and xbar transposes drop their DMA-lane waits (the
    Tile scheduler models all DMA as one exclusive device and would
    serialize the streams; they live on separate SWDGE/HWDGE queues and
    write fresh tiles, so FIFO order per queue is enough). Each transpose
    is rewired to the exact DVE-sem value of its own casts (the
    scheduler's thresholds are stale once the chains are stripped). Reads
    are paced 3 groups behind the transposes: an unthrottled read stream
    starves xbar packets ~15x (2KB read packets vs 256B xbar packets in
    the SDMA round-robin). End-block rendezvous waits are deduped.
"""

import numpy as np

B, S, H = 32, 4096, 256
NCORES = 8
BL = B // NCORES  # batches per core
NG = 8            # s-groups of 512 rows
E = H

_CACHE = {}


def _split_multiwait(nc, mybir):
    """This walrus/ISA build allows ONE sync-wait slot per instruction.
    Move extra waits onto same-engine NoOps inserted just before."""
    for blk in nc.m.functions[0].blocks:
        insts = blk.instructions
        out = []
        changed = False
        for inst in insts:
            si = inst.sync_info
            waits = list(si.on_wait) if si is not None else []
            if len(waits) > 1:
                for w in waits[:-1]:
                    nop = mybir.InstNoOp(
                        name=nc.get_next_instruction_name(), ins=[], outs=[]
                    )
                    nop.engine = inst.engine
                    nop.sync_info = mybir.SyncInfo(on_wait=[w], on_update=[])
                    out.append(nop)
                inst.sync_info = mybir.SyncInfo(
                    on_wait=[waits[-1]], on_update=list(si.on_update)
                )
                changed = True
            out.append(inst)
        if changed:
            insts[:] = out


def _dma_surgery(nc, mybir, read_names, tpose_deps, setup_read_names=(),
                 pace_deps=None, verbose=False):
    """Drop DMA-lane waits from the SWDGE reads and HWDGE xbar transposes
    so the two streams pipeline instead of serializing on the scheduler's
    exclusive-DMA-device sem chains. Each transpose's wait is REWRITTEN to
    the exact cumulative DVE-sem value of its own group's last cast: the
    scheduler's original DVE thresholds were computed against its
    serialized schedule and reference much later DVE instructions once
    the DMA chains are stripped.
    """
    blocks = nc.m.functions[0].blocks
    insts = {}
    order = []
    for blk in blocks:
        for i in blk.instructions:
            insts[i.name] = i
            order.append(i)

    # cumulative DVE engine-sem value after each cast instruction, and
    # cumulative DMA-lane value after each transpose (for pacing)
    cum = {}
    cast_sig = {}
    tpose_sig = {}
    cast_set = {cn for _, cns in tpose_deps for cn in cns}
    tset_all = {tn for tn, _ in tpose_deps}
    for i in order:
        si = i.sync_info
        if si is None:
            continue
        for u in si.on_update:
            if u.ant_name.startswith("DVE"):
                cum[u.ant_name] = cum.get(u.ant_name, 0) + u.update_value
                if i.name in cast_set:
                    cast_sig[i.name] = (u, cum[u.ant_name])
            elif u.ant_name.startswith("DMAHW") or u.ant_name.startswith("DMASW"):
                cum[u.ant_name] = cum.get(u.ant_name, 0) + u.update_value
                if i.name in tset_all:
                    tpose_sig[i.name] = (u, cum[u.ant_name])

    def is_lane(w):
        return w.ant_name.startswith("DMASW") or w.ant_name.startswith("DMAHW")

    for rn in list(read_names) + list(setup_read_names):
        i = insts[rn]
        si = i.sync_info
        if si is None:
            continue
        keep = [w for w in si.on_wait if not is_lane(w)]
        if verbose and len(keep) != len(si.on_wait):
            print(f"  read {rn}: dropped {len(si.on_wait) - len(keep)} lane waits")
        i.sync_info = mybir.SyncInfo(on_wait=keep, on_update=list(si.on_update))

    # Pacing: a read may start only once the transpose two groups back has
    # completed, so at most one in-flight HBM read competes with each xbar
    # transpose for SDMA packet slots (reads' ~8x larger packets would
    # otherwise starve the transposes ~15x, measured).
    for rn, tns in (pace_deps or {}).items():
        i = insts[rn]
        si = i.sync_info
        waits = list(si.on_wait) if si else []
        for tn in tns:
            u, val = tpose_sig[tn]
            waits.append(
                mybir.SyncWait(
                    sync_type="semaphore",
                    id=u.id,
                    ant_name=u.ant_name,
                    wait_mode="sem-ge-imm",
                    wait_value=val,
                )
            )
        if verbose:
            print(f"  pace read {rn}: += {tns}")
        i.sync_info = mybir.SyncInfo(
            on_wait=waits, on_update=list(si.on_update) if si else []
        )

    for tn, cast_names in tpose_deps:
        i = insts[tn]
        si = i.sync_info
        best = None
        for cn in cast_names:
            if cn in cast_sig:
                u, val = cast_sig[cn]
                if best is None or val > best[1]:
                    best = (u, val)
        waits = []
        if best is not None:
            u, val = best
            waits.append(
                mybir.SyncWait(
                    sync_type="semaphore",
                    id=u.id,
                    ant_name=u.ant_name,
                    wait_mode="sem-ge-imm",
                    wait_value=val,
                )
            )
        else:
            # fall back: keep non-lane waits
            waits = [w for w in (si.on_wait if si else []) if not is_lane(w)]
        if verbose:
            old = [f"{w.ant_name}>={w.wait_value}" for w in (si.on_wait if si else [])]
            print(f"  tpose {tn}: {old} -> "
                  f"{[f'{w.ant_name}>={w.wait_value}' for w in waits]}")
        i.sync_info = mybir.SyncInfo(
            on_wait=waits, on_update=list(si.on_update) if si else []
        )

    # End-block rendezvous dedupe (one waiter per (sem, value) suffices).
    for blk in blocks:
        if not (blk.name.startswith("tile_context") and blk.name.endswith("_end")):
            continue
        seen = set()
        for i in blk.instructions:
            si = i.sync_info
            if si is None or not si.on_wait:
                continue
            keep = []
            for w in si.on_wait:
                if w.wait_mode != "sem-ge-imm" or "barrier" in w.ant_name:
                    keep.append(w)
                    continue
                key = (w.ant_name, w.wait_value)
                if key not in seen:
                    seen.add(key)
                    keep.append(w)
            if len(keep) != len(si.on_wait):
                i.sync_info = mybir.SyncInfo(
                    on_wait=keep, on_update=list(si.on_update)
                )

    # Scheduler-issued SP-side read waits would head-of-line block the
    # Sync FIFO (which issues the xbar transposes); drop pure-lane waits
    # on non-DMA SP instructions.
    tset = {tn for tn, _ in tpose_deps}
    for blk in blocks:
        if not (blk.name.startswith("tile_context") and not blk.name.endswith("_end")):
            continue
        for i in blk.instructions:
            if i.engine != mybir.EngineType.SP or i.name in tset:
                continue
            if isinstance(i, (mybir.InstDMACopy, mybir.InstDmaTransposeAnt)):
                continue
            si = i.sync_info
            if si is None or not si.on_wait:
                continue
            if all(w.ant_name.startswith("DMASW") for w in si.on_wait):
                i.sync_info = mybir.SyncInfo(on_wait=[], on_update=list(si.on_update))


def _build(verbose=False):
    import concourse.bass as bass
    import concourse.tile as tile
    from concourse import mybir
    from concourse.masks import make_identity

    f32 = mybir.dt.float32
    bf16 = mybir.dt.bfloat16
    fp8 = mybir.dt.float8e4
    u16 = mybir.dt.uint16
    AF = mybir.ActivationFunctionType

    nc = bass.Bass(num_swdge_queues=2, dynamic_dma_scratch_size=65536)
    hid_t = nc.dram_tensor("hidden", [BL, H], f32, kind="ExternalInput")
    enc_t = nc.dram_tensor("enc", [S, BL, H], f32, kind="ExternalInput")
    wat_t = nc.dram_tensor("w_attn", [H, 2 * H], f32, kind="ExternalInput")
    bat_t = nc.dram_tensor("b_attn", [H], f32, kind="ExternalInput")
    wv_t = nc.dram_tensor("w_v", [1, H], f32, kind="ExternalInput")
    ctxu_t = nc.dram_tensor("ctxu", [2, 2, 512], f32, kind="ExternalOutput")
    den_t = nc.dram_tensor("den", [97, 1], f32, kind="ExternalOutput")

    hid = hid_t.ap()
    enc = enc_t.ap()
    wat = wat_t.ap()
    bat = bat_t.ap().rearrange("(o c) -> o c", o=1)  # [1, 256]
    wv = wv_t.ap()

    read_names = []
    setup_read_names = []
    tpose_deps = []        # (transpose name, [cast instr names])
    read_of_group = {}     # g -> [read names]
    tpose_of_group = {}    # g -> [transpose names]


    with tile.TileContext(nc) as tc:
        with (
            tc.tile_pool(name="const", bufs=1) as cp,
            tc.tile_pool(name="xres", bufs=1) as xrp,
            tc.tile_pool(name="x8", bufs=1) as x8p,
            tc.tile_pool(name="xt8", bufs=1) as xtp,
            tc.tile_pool(name="thp", bufs=8) as thp,
            tc.tile_pool(name="stat", bufs=1) as stp,
            tc.tile_pool(name="misc", bufs=2) as wp,
            tc.tile_pool(name="pe", bufs=5, space="PSUM") as ppe,
            tc.tile_pool(name="ps", bufs=2, space="PSUM") as pps,
            tc.tile_pool(name="pc", bufs=1, space="PSUM") as ppc,
        ):
            ident = cp.tile([128, 128], f32)
            make_identity(nc, ident)
            ident16 = cp.tile([128, 128], bf16)
            nc.vector.tensor_copy(out=ident16, in_=ident)

            # ---------- resident enc: bf16 cast-DMA reads ----------
            x_res = []
            for g in range(NG):
                t = xrp.tile([128, 4, 4 * H], bf16, tag=f"xr{g}", name=f"xr{g}")
                src = enc[g * 512 : (g + 1) * 512, :, :].rearrange(
                    "(p i) b h -> p i (b h)", i=4
                )
                # 1-MiB i-halves for every group: group g's transpose of
                # half 0 overlaps its own half-1 read, cutting per-group
                # delivery latency.
                r0 = nc.gpsimd.dma_start(out=t[:, 0:2], in_=src[:, 0:2])
                r1 = nc.gpsimd.dma_start(out=t[:, 2:4], in_=src[:, 2:4])
                read_names += [r0.ins.name, r1.ins.name]
                read_of_group[g] = [r0.ins.name, r1.ins.name]
                x_res.append(t)

            st_g = [
                stp.tile([97, 512], bf16, tag=f"st{g}", name=f"st{g}")
                for g in range(NG)
            ]
            for g in range(NG):
                nc.vector.memset(st_g[g], 0.0)

            # fp8 X with interleaved h (h' = 2k + hh), free layout (i, b, h')
            x8 = [
                x8p.tile([128, 4096], fp8, tag=f"x8{g}", name=f"x8{g}")
                for g in range(NG)
            ]
            # transposed fp8 X^T: [128 p(k), 16 blk(i,b), 128 s, 2 hh] via u16 xbar
            xt8 = [
                xtp.tile([128, 4096], fp8, tag=f"xt{g}", name=f"xt{g}")
                for g in range(NG)
            ]

            u_g = [
                stp.tile([128, BL, 4], bf16, tag=f"ug{g}", name=f"ug{g}")
                for g in range(NG)
            ]
            acc_all = stp.tile([97, NG], f32)
            # DoubleRow fp8 We^T, split into value + residual terms (both
            # pre-scaled x32; the tanh descales by 1/32). Layout [k, hh, e]:
            # plane hh matches the pair-interleaved X^T (h = hh*128 + k).
            wet8a = cp.tile([128, 2, E], fp8, tag="wet8a", name="wet8a")
            wet8b = cp.tile([128, 2, E], fp8, tag="wet8b", name="wet8b")
            qb = [cp.tile([128, BL], f32, tag=f"qb{i}", name=f"qb{i}") for i in range(2)]
            vt16 = [cp.tile([128, 1], bf16, tag=f"vt{i}", name=f"vt{i}") for i in range(2)]

            # ---------------- setup: weights / q / v ----------------
            with tc.tile_pool(name="setsb", bufs=1) as ssb:
                w_nat2 = ssb.tile([128, 2, 2 * H], f32, tag="wn", name="wn")
                rw = nc.sync.dma_start(
                    out=w_nat2,
                    in_=wat.rearrange("(eh p) c -> p eh c", eh=2),
                )
                setup_read_names.append(rw.ins.name)
                w_nat = [w_nat2[:, i] for i in range(2)]
                b_attn_sb = ssb.tile([1, H], f32)
                rb = nc.sync.dma_start(out=b_attn_sb, in_=bat)
                setup_read_names.append(rb.ins.name)
                v_sb = ssb.tile([1, H], f32)
                rv = nc.sync.dma_start(out=v_sb, in_=wv)
                setup_read_names.append(rv.ins.name)
                h_nat = ssb.tile([BL, H], f32)
                rh = nc.sync.dma_start(out=h_nat, in_=hid)
                setup_read_names.append(rh.ins.name)
                ones4 = ssb.tile([1, BL], f32)
                nc.vector.memset(ones4, 1.0)

                wht = [
                    ssb.tile([128, E], f32, tag=f"wht{i}", name=f"wht{i}")
                    for i in range(2)
                ]
                for eh in range(2):
                    for cblk in range(4):  # column blocks of W_attn
                        pt = pps.tile([128, 128], f32, tag="s", bufs=2, name="pt_w")
                        nc.tensor.transpose(
                            pt, w_nat[eh][:, cblk * 128 : (cblk + 1) * 128], ident
                        )
                        if cblk < 2:  # Wh columns
                            nc.scalar.copy(
                                out=wht[cblk][:, eh * 128 : (eh + 1) * 128], in_=pt
                            )
                        else:  # We columns -> fp8 value + residual terms
                            hh = cblk - 2
                            sl = slice(eh * 128, (eh + 1) * 128)
                            nc.scalar.activation(
                                out=wet8a[:, hh, sl], in_=pt, func=AF.Copy,
                                scale=32.0,
                            )
                            nc.vector.scalar_tensor_tensor(
                                out=wet8b[:, hh, sl],
                                in0=pt, scalar=32.0, in1=wet8a[:, hh, sl],
                                op0=mybir.AluOpType.mult,
                                op1=mybir.AluOpType.subtract,
                            )

                ht = [
                    ssb.tile([128, BL], f32, tag=f"ht{i}", name=f"ht{i}")
                    for i in range(2)
                ]
                for hh in range(2):
                    pt = pps.tile([128, 128], f32, tag="s", bufs=2, name="pt_h")
                    nc.tensor.transpose(
                        pt[:, :BL], h_nat[:, hh * 128 : (hh + 1) * 128], ident[:BL, :BL]
                    )
                    nc.scalar.copy(out=ht[hh], in_=pt[:, :BL])

                for eh in range(2):
                    pt = pps.tile([128, 128], f32, tag="s", bufs=2, name="pt_v")
                    nc.tensor.transpose(
                        pt[:, :1], v_sb[:, eh * 128 : (eh + 1) * 128], ident[:1, :1]
                    )
                    nc.scalar.copy(out=vt16[eh], in_=pt[:, :1])

                # qb[eh][e, b] = sum_h WhT[h, e] * hT[h, b] + b_attn[e]
                for eh in range(2):
                    pq = pps.tile([128, 128], f32, tag="s", bufs=2, name="pt_q")
                    for hh in range(2):
                        nc.tensor.matmul(
                            pq[:, :BL],
                            wht[hh][:, eh * 128 : (eh + 1) * 128],
                            ht[hh],
                            start=(hh == 0),
                            stop=False,
                        )
                    nc.tensor.matmul(
                        pq[:, :BL],
                        b_attn_sb[:, eh * 128 : (eh + 1) * 128],
                        ones4,
                        start=False,
                        stop=True,
                    )
                    nc.scalar.copy(out=qb[eh], in_=pq[:, :BL])

            # ---------------- main loop ----------------
            # both context halves share one PSUM bank: half h lives at
            # partition rows {32h, 32h+1} (tile_position col offsets), which
            # frees a bank for a 5th energy buffer.
            pctx_bank = ppc.tile([34, 512], f32, tag="ctx", name="pctx")
            pctx = [pctx_bank[32 * h : 32 * h + 2] for h in range(2)]

            def ctx_group(g):
                for half in range(2):
                    for jl in range(4):
                        n = g * 4 + jl
                        nc.tensor.matmul(
                            pctx[half],
                            u_g[g][:, 2 * half : 2 * half + 2, jl],
                            x_res[g][:, jl, half * 512 : (half + 1) * 512],
                            start=(n == 0),
                            stop=(n == NG * 4 - 1),
                            tile_position=(0, 32 * half),
                        )

            def strip_block(g):
                # exp strips of group g -> u_g[g] (PE transposes + DVE copies)
                for c in range(4):
                    pt = pps.tile([128, 256], bf16, tag="s", bufs=2, name="pt_u")
                    nc.tensor.transpose(
                        pt[:, :97],
                        st_g[g][:, c * 128 : (c + 1) * 128],
                        ident16[:97, :97],
                    )
                    nc.vector.tensor_copy(
                        out=u_g[g][:, :, c],
                        in_=pt.rearrange("p (a r) -> p a r", r=32)[:, :4, 0],
                    )

            def cast_group(g, ihalf=None):
                # x_res[g] (i-slice) -> x8[g] with h-halves interleaved.
                ii = range(4) if ihalf is None else range(2 * ihalf, 2 * ihalf + 2)
                names = []
                for i in ii:
                    src = x_res[g][:, i].rearrange("p (b hh k) -> p b hh k", b=4, hh=2)
                    dst = x8[g].rearrange(
                        "p (i b k hh) -> p i b hh k", i=4, b=4, hh=2
                    )[:, i]
                    cc = nc.vector.tensor_copy(out=dst, in_=src)
                    names.append(cc.ins.name)
                return names

            def tpose_group(g, ihalf=None):
                # fp8-pair xbar transpose of x8[g] (i-slice) into xt8[g].
                # Transport dtype is bf16 (bitcast): the xbar moves 16-bit
                # elements regardless, and the bf16-typed path runs ~2x the
                # element rate of uint16 in practice.
                if ihalf is None:
                    src = x8[g].bitcast(bf16)
                    dst = xt8[g].bitcast(bf16).rearrange("p (grp s) -> p grp s", s=128)
                else:
                    src = x8[g][:, 2048 * ihalf : 2048 * (ihalf + 1)].bitcast(bf16)
                    dst = xt8[g][:, 2048 * ihalf : 2048 * (ihalf + 1)].bitcast(
                        bf16
                    ).rearrange("p (grp s) -> p grp s", s=128)
                return nc.sync.dma_start_transpose(dst, src)

            # fp8 X^T DoubleRow views: rhs(g, b) = [128 p, 2 hh, 4 i, 128 s];
            # the hh pair is fp8-adjacent (stride 1), so the DoubleRow ifmap
            # fetch reads contiguous 16-bit pairs at 2 rows/cycle.
            def rhs_view(g, b):
                v = xt8[g].rearrange(
                    "p (i b s hh) -> p b hh i s", i=4, b=4, hh=2
                )
                return v[:, b]

            for g in range(NG):
                for ih in range(2):
                    cnames = cast_group(g, ih)
                    tp = tpose_group(g, ih)
                    tpose_deps.append((tp.ins.name, cnames))
                    tpose_of_group.setdefault(g, []).append(tp.ins.name)

                strip = pps.tile([97, 512], f32, tag="s", name="strip")

                def energy_block(eh):
                    pe_t = [
                        ppe.tile([128, 512], f32, tag="e", name=f"pe{b}")
                        for b in range(BL)
                    ]
                    for term in range(2):
                        wt = wet8a if term == 0 else wet8b
                        for b in range(BL):
                            nc.tensor.matmul(
                                pe_t[b],
                                wt[:, :, eh * 128 : (eh + 1) * 128],
                                rhs_view(g, b),
                                start=(term == 0),
                                stop=(term == 1),
                                perf_mode=mybir.MatmulPerfMode.DoubleRow,
                            )
                    th_eh = []
                    for b in range(BL):
                        th = thp.tile([128, 512], bf16, tag="th", name="th")
                        nc.scalar.activation(
                            out=th,
                            in_=pe_t[b],
                            func=AF.Tanh,
                            bias=qb[eh][:, b : b + 1],
                            scale=1.0 / 32.0,
                        )
                        th_eh.append(th)
                    return th_eh

                def vdot_block(eh, th_eh):
                    for b in range(BL):
                        nc.tensor.matmul(
                            strip[32 * b : 32 * b + 1, :],
                            vt16[eh],
                            th_eh[b],
                            start=(eh == 0),
                            stop=(eh == 1),
                            tile_position=(0, 32 * b),
                        )

                # The previous group's strip transposes + ctx first: they
                # fill the PE while this group's transpose lands, and the
                # 4-deep PSUM pool chains energy(g) behind tanh(g-1) anyway.
                if g >= 1:
                    strip_block(g - 1)
                    ctx_group(g - 1)
                th0 = energy_block(0)
                vdot_block(0, th0)
                th1 = energy_block(1)
                vdot_block(1, th1)

                nc.scalar.activation(
                    out=st_g[g],
                    in_=strip,
                    func=AF.Exp,
                    accum_out=acc_all[:, g : g + 1],
                )

            strip_block(NG - 1)

            # denominators ship first so their HBM-write receipt hides
            # under the context tail
            accs = wp.tile([97, 1], f32)
            nc.vector.reduce_sum(out=accs, in_=acc_all, axis=mybir.AxisListType.X)
            nc.sync.dma_start(out=den_t.ap(), in_=accs)

            g = NG - 1
            for half in range(2):
                for jl in range(4):
                    n = g * 4 + jl
                    nc.tensor.matmul(
                        pctx[half],
                        u_g[g][:, 2 * half : 2 * half + 2, jl],
                        x_res[g][:, jl, half * 512 : (half + 1) * 512],
                        start=(n == 0),
                        stop=(n == NG * 4 - 1),
                        tile_position=(0, 32 * half),
                    )
                csb = wp.tile([2, 512], f32, tag="csb", name=f"csb{half}")
                nc.scalar.copy(out=csb, in_=pctx[half])
                nc.sync.dma_start(out=ctxu_t.ap()[half], in_=csb)

    pace_deps = {
        read_of_group[g][0]: [tpose_of_group[g - 3][1]] for g in range(3, NG)
    }
    _dma_surgery(nc, mybir, read_names, tpose_deps, setup_read_names,
                 pace_deps=pace_deps, verbose=verbose)
    _split_multiwait(nc, mybir)
    return nc


def kernel(**inputs):
    from concourse.bass_utils import run_bass_kernel_spmd

    hidden = np.asarray(inputs["hidden"], dtype=np.float32)
    enc = np.asarray(inputs["encoder_outputs"], dtype=np.float32)
    w_attn = np.ascontiguousarray(np.asarray(inputs["W_attn"], dtype=np.float32))
    b_attn = np.ascontiguousarray(np.asarray(inputs["b_attn"], dtype=np.float32))
    w_v = np.ascontiguousarray(np.asarray(inputs["W_v"], dtype=np.float32))

    if "nc" not in _CACHE:
        _CACHE["nc"] = _build()
    nc = _CACHE["nc"]

    in_maps = []
    for c in range(NCORES):
        sl = slice(c * BL, (c + 1) * BL)
        in_maps.append(
            {
                "hidden": np.ascontiguousarray(hidden[sl]),
                "enc": np.ascontiguousarray(enc[:, sl, :]),
                "w_attn": w_attn,
                "b_attn": b_attn,
                "w_v": w_v,
            }
        )

    trace = bool(_CACHE.get("trace", False))
    res = run_bass_kernel_spmd(nc, in_maps, core_ids=list(range(NCORES)), trace=trace)
    _CACHE["last_results"] = res

    out = np.empty((1, B, H), dtype=np.float32)
    for c in range(NCORES):
        ctxu = res.results[c]["ctxu"]  # [2, 2, 512]
        den = res.results[c]["den"]    # [97, 1]
        for b in range(BL):
            half, row = b // 2, b % 2
            vals = ctxu[half, row, row * 256 : row * 256 + 256]
            out[0, c * BL + b] = vals / den[32 * b, 0]
    return out


# revision 29
# speedup vs baseline: 1.1005x; 1.1005x over previous
"""Bahdanau-attention kernel for trn2, data-parallel over batch across 8 cores.

Per-core computation (B_LOC = 4 batches, S = 4096, H = E = 256):
  energy = tanh(hidden @ Wh.T + enc @ We.T + b_attn)      [b, s, e]
  scores = energy . v                                      [b, s]
  attn   = softmax(scores) over s  (no max-subtraction: scores bounded by ||v||_1)
  out    = sum_s attn * enc                                [b, h]

Design (fp8-pair xbar transposes + DoubleRow split-We energy):
  - enc slice read from HBM once via SWDGE cast-DMA (f32 -> bf16) in
    1-MiB i-halves, 4 consecutive s-rows per partition (16KB-contiguous
    read descriptors). The SDMA system moves ~460 GB/s summed over both
    sides of every transfer, so total DMA bytes are the budget that
    matters: this design moves 33.6 MB vs v1's 41.8 MB.
  - The energy matmul needs X^T (h on partitions). DVE casts X to fp8e4
    with the two h-halves INTERLEAVED along the free dim (h' = 2k + hh);
    a bf16-bitcast xbar transpose then moves 16-bit h-PAIRS: partition k
    of the output holds (h=k, h=128+k) interleaved along free — exactly
    the two k-tile planes of a DoubleRow matmul. Transpose bytes drop 4x
    vs transposing bf16 X directly.
  - Energy: fp8 DoubleRow matmuls (full 256-deep contraction per MM, the
    interleaved pairs fetched as contiguous 16-bit reads at 2 rows/cyc).
    fp8 We alone costs 1.7e-2 rel (host-sim) — so We is SPLIT into
    value + residual fp8 terms (both pre-scaled x32, descaled by the
    tanh's scale=1/32), which recovers ~bf16 weight accuracy; fp8 X
    contributes the remaining ~5.5e-3.
  - tanh folds the per-partition bias qb[e] = hidden @ Wh.T + b_attn; the
    v-dot runs on the PE with v stationary, 4 batches' score strips in one
    PSUM bank at partitions {0,32,64,96} (tile_position col packing).
  - One Exp per group produces exp(scores) + denominators (accum_out);
    exp strips are PE-transposed to [s-part, b]; the unnormalized context
    accumulates in ONE shared PSUM bank (halves at row offsets 0/32 via
    tile_position) against the resident native bf16 X (full-precision
    path), freeing a bank for a 5-deep energy-PSUM pool. Softmax division
    happens on the host.
  - DMA surgery: reads and xbar transposes drop their DMA-lane waits (the
    Tile scheduler models all DMA as one exclusive device and would
    serialize the streams; they live on separate SWDGE/HWDGE queues and
    write fresh tiles, so FIFO order per queue is enough). Each transpose
    is rewired to the exact DVE-sem value of its own casts (the
    scheduler's thresholds are stale once the chains are stripped). Reads
    are paced 3 groups behind the transposes: an unthrottled read stream
    starves xbar packets ~15x (2KB read packets vs 256B xbar packets in
    the SDMA round-robin). End-block rendezvous waits are deduped.
"""

import numpy as np

B, S, H = 32, 4096, 256
NCORES = 8
BL = B // NCORES  # batches per core
NG = 8            # s-groups of 512 rows
E = H

_CACHE = {}


def _split_multiwait(nc, mybir):
    """This walrus/ISA build allows ONE sync-wait slot per instruction.
    Move extra waits onto same-engine NoOps inserted just before."""
    for blk in nc.m.functions[0].blocks:
        insts = blk.instructions
        out = []
        changed = False
        for inst in insts:
            si = inst.sync_info
            waits = list(si.on_wait) if si is not None else []
            if len(waits) > 1:
                for w in waits[:-1]:
                    nop = mybir.InstNoOp(
                        name=nc.get_next_instruction_name(), ins=[], outs=[]
                    )
                    nop.engine = inst.engine
                    nop.sync_info = mybir.SyncInfo(on_wait=[w], on_update=[])
                    out.append(nop)
                inst.sync_info = mybir.SyncInfo(
                    on_wait=[waits[-1]], on_update=list(si.on_update)
                )
                changed = True
            out.append(inst)
        if changed:
            insts[:] = out


def _dma_surgery(nc, mybir, read_names, tpose_deps, setup_read_names=(),
                 pace_deps=None, verbose=False):
    """Drop DMA-lane waits from the SWDGE reads and HWDGE xbar transposes
    so the two streams pipeline instead of serializing on the scheduler's
    exclusive-DMA-device sem chains. Each transpose's wait is REWRITTEN to
    the exact cumulative DVE-sem value of its own group's last cast: the
    scheduler's original DVE thresholds were computed against its
    serialized schedule and reference much later DVE instructions once
    the DMA chains are stripped.
    """
    blocks = nc.m.functions[0].blocks
    insts = {}
    order = []
    for blk in blocks:
        for i in blk.instructions:
            insts[i.name] = i
            order.append(i)

    # cumulative DVE engine-sem value after each cast instruction, and
    # cumulative DMA-lane value after each transpose (for pacing)
    cum = {}
    cast_sig = {}
    tpose_sig = {}
    cast_set = {cn for _, cns in tpose_deps for cn in cns}
    tset_all = {tn for tn, _ in tpose_deps}
    for i in order:
        si = i.sync_info
        if si is None:
            continue
        for u in si.on_update:
            if u.ant_name.startswith("DVE"):
                cum[u.ant_name] = cum.get(u.ant_name, 0) + u.update_value
                if i.name in cast_set:
                    cast_sig[i.name] = (u, cum[u.ant_name])
            elif u.ant_name.startswith("DMAHW") or u.ant_name.startswith("DMASW"):
                cum[u.ant_name] = cum.get(u.ant_name, 0) + u.update_value
                if i.name in tset_all:
                    tpose_sig[i.name] = (u, cum[u.ant_name])

    def is_lane(w):
        return w.ant_name.startswith("DMASW") or w.ant_name.startswith("DMAHW")

    for rn in list(read_names) + list(setup_read_names):
        i = insts[rn]
        si = i.sync_info
        if si is None:
            continue
        keep = [w for w in si.on_wait if not is_lane(w)]
        if verbose and len(keep) != len(si.on_wait):
            print(f"  read {rn}: dropped {len(si.on_wait) - len(keep)} lane waits")
        i.sync_info = mybir.SyncInfo(on_wait=keep, on_update=list(si.on_update))

    # Pacing: a read may start only once the transpose two groups back has
    # completed, so at most one in-flight HBM read competes with each xbar
    # transpose for SDMA packet slots (reads' ~8x larger packets would
    # otherwise starve the transposes ~15x, measured).
    for rn, tns in (pace_deps or {}).items():
        i = insts[rn]
        si = i.sync_info
        waits = list(si.on_wait) if si else []
        for tn in tns:
            u, val = tpose_sig[tn]
            waits.append(
                mybir.SyncWait(
                    sync_type="semaphore",
                    id=u.id,
                    ant_name=u.ant_name,
                    wait_mode="sem-ge-imm",
                    wait_value=val,
                )
            )
        if verbose:
            print(f"  pace read {rn}: += {tns}")
        i.sync_info = mybir.SyncInfo(
            on_wait=waits, on_update=list(si.on_update) if si else []
        )

    for tn, cast_names in tpose_deps:
        i = insts[tn]
        si = i.sync_info
        best = None
        for cn in cast_names:
            if cn in cast_sig:
                u, val = cast_sig[cn]
                if best is None or val > best[1]:
                    best = (u, val)
        waits = []
        if best is not None:
            u, val = best
            waits.append(
                mybir.SyncWait(
                    sync_type="semaphore",
                    id=u.id,
                    ant_name=u.ant_name,
                    wait_mode="sem-ge-imm",
                    wait_value=val,
                )
            )
        else:
            # fall back: keep non-lane waits
            waits = [w for w in (si.on_wait if si else []) if not is_lane(w)]
        if verbose:
            old = [f"{w.ant_name}>={w.wait_value}" for w in (si.on_wait if si else [])]
            print(f"  tpose {tn}: {old} -> "
                  f"{[f'{w.ant_name}>={w.wait_value}' for w in waits]}")
        i.sync_info = mybir.SyncInfo(
            on_wait=waits, on_update=list(si.on_update) if si else []
        )

    # End-block rendezvous dedupe (one waiter per (sem, value) suffices).
    for blk in blocks:
        if not (blk.name.startswith("tile_context") and blk.name.endswith("_end")):
            continue
        seen = set()
        for i in blk.instructions:
            si = i.sync_info
            if si is None or not si.on_wait:
                continue
            keep = []
            for w in si.on_wait:
                if w.wait_mode != "sem-ge-imm" or "barrier" in w.ant_name:
                    keep.append(w)
                    continue
                key = (w.ant_name, w.wait_value)
                if key not in seen:
                    seen.add(key)
                    keep.append(w)
            if len(keep) != len(si.on_wait):
                i.sync_info = mybir.SyncInfo(
                    on_wait=keep, on_update=list(si.on_update)
                )

    # Scheduler-issued SP-side read waits would head-of-line block the
    # Sync FIFO (which issues the xbar transposes); drop pure-lane waits
    # on non-DMA SP instructions.
    tset = {tn for tn, _ in tpose_deps}
    for blk in blocks:
        if not (blk.name.startswith("tile_context") and not blk.name.endswith("_end")):
            continue
        for i in blk.instructions:
            if i.engine != mybir.EngineType.SP or i.name in tset:
                continue
            if isinstance(i, (mybir.InstDMACopy, mybir.InstDmaTransposeAnt)):
                continue
            si = i.sync_info
            if si is None or not si.on_wait:
                continue
            if all(w.ant_name.startswith("DMASW") for w in si.on_wait):
                i.sync_info = mybir.SyncInfo(on_wait=[], on_update=list(si.on_update))


def _build(verbose=False):
    import concourse.bass as bass
    import concourse.tile as tile
    from concourse import mybir
    from concourse.masks import make_identity

    f32 = mybir.dt.float32
    bf16 = mybir.dt.bfloat16
    fp8 = mybir.dt.float8e4
    u16 = mybir.dt.uint16
    AF = mybir.ActivationFunctionType

    nc = bass.Bass(num_swdge_queues=2, dynamic_dma_scratch_size=65536)
    hid_t = nc.dram_tensor("hidden", [BL, H], f32, kind="ExternalInput")
    enc_t = nc.dram_tensor("enc", [S, BL, H], f32, kind="ExternalInput")
    wat_t = nc.dram_tensor("w_attn", [H, 2 * H], f32, kind="ExternalInput")
    bat_t = nc.dram_tensor("b_attn", [H], f32, kind="ExternalInput")
    wv_t = nc.dram_tensor("w_v", [1, H], f32, kind="ExternalInput")
    ctxu_t = nc.dram_tensor("ctxu", [2, 2, 512], f32, kind="ExternalOutput")
    den_t = nc.dram_tensor("den", [97, 1], f32, kind="ExternalOutput")

    hid = hid_t.ap()
    enc = enc_t.ap()
    wat = wat_t.ap()
    bat = bat_t.ap().rearrange("(o c) -> o c", o=1)  # [1, 256]
    wv = wv_t.ap()

    read_names = []
    setup_read_names = []
    tpose_deps = []        # (transpose name, [cast instr names])
    read_of_group = {}     # g -> [read names]
    tpose_of_group = {}    # g -> [transpose names]


    with tile.TileContext(nc) as tc:
        with (
            tc.tile_pool(name="const", bufs=1) as cp,
            tc.tile_pool(name="xres", bufs=1) as xrp,
            tc.tile_pool(name="x8", bufs=1) as x8p,
            tc.tile_pool(name="xt8", bufs=1) as xtp,
            tc.tile_pool(name="thp", bufs=8) as thp,
            tc.tile_pool(name="stat", bufs=1) as stp,
            tc.tile_pool(name="misc", bufs=2) as wp,
            tc.tile_pool(name="pe", bufs=5, space="PSUM") as ppe,
            tc.tile_pool(name="ps", bufs=2, space="PSUM") as pps,
            tc.tile_pool(name="pc", bufs=1, space="PSUM") as ppc,
        ):
            ident = cp.tile([128, 128], f32)
            make_identity(nc, ident)
            ident16 = cp.tile([128, 128], bf16)
            nc.vector.tensor_copy(out=ident16, in_=ident)

            # ---------- resident enc: bf16 cast-DMA reads ----------
            x_res = []
            for g in range(NG):
                t = xrp.tile([128, 4, 4 * H], bf16, tag=f"xr{g}", name=f"xr{g}")
                src = enc[g * 512 : (g + 1) * 512, :, :].rearrange(
                    "(p i) b h -> p i (b h)", i=4
                )
                # 1-MiB i-halves for every group: group g's transpose of
                # half 0 overlaps its own half-1 read, cutting per-group
                # delivery latency.
                r0 = nc.gpsimd.dma_start(out=t[:, 0:2], in_=src[:, 0:2])
                r1 = nc.gpsimd.dma_start(out=t[:, 2:4], in_=src[:, 2:4])
                read_names += [r0.ins.name, r1.ins.name]
                read_of_group[g] = [r0.ins.name, r1.ins.name]
                x_res.append(t)

            st_g = [
                stp.tile([97, 512], bf16, tag=f"st{g}", name=f"st{g}")
                for g in range(NG)
            ]
            for g in range(NG):
                nc.vector.memset(st_g[g], 0.0)

            # fp8 X with interleaved h (h' = 2k + hh), free layout (i, b, h')
            x8 = [
                x8p.tile([128, 4096], fp8, tag=f"x8{g}", name=f"x8{g}")
                for g in range(NG)
            ]
            # transposed fp8 X^T: [128 p(k), 16 blk(i,b), 128 s, 2 hh] via u16 xbar
            xt8 = [
                xtp.tile([128, 4096], fp8, tag=f"xt{g}", name=f"xt{g}")
                for g in range(NG)
            ]

            u_g = [
                stp.tile([128, BL, 4], bf16, tag=f"ug{g}", name=f"ug{g}")
                for g in range(NG)
            ]
            acc_all = stp.tile([97, NG], f32)
            # DoubleRow fp8 We^T, split into value + residual terms (both
            # pre-scaled x32; the tanh descales by 1/32). Layout [k, hh, e]:
            # plane hh matches the pair-interleaved X^T (h = hh*128 + k).
            wet8a = cp.tile([128, 2, E], fp8, tag="wet8a", name="wet8a")
            wet8b = cp.tile([128, 2, E], fp8, tag="wet8b", name="wet8b")
            qb = [cp.tile([128, BL], f32, tag=f"qb{i}", name=f"qb{i}") for i in range(2)]
            vt16 = [cp.tile([128, 1], bf16, tag=f"vt{i}", name=f"vt{i}") for i in range(2)]

            # ---------------- setup: weights / q / v ----------------
            with tc.tile_pool(name="setsb", bufs=1) as ssb:
                w_nat2 = ssb.tile([128, 2, 2 * H], f32, tag="wn", name="wn")
                rw = nc.sync.dma_start(
                    out=w_nat2,
                    in_=wat.rearrange("(eh p) c -> p eh c", eh=2),
                )
                setup_read_names.append(rw.ins.name)
                w_nat = [w_nat2[:, i] for i in range(2)]
                b_attn_sb = ssb.tile([1, H], f32)
                rb = nc.sync.dma_start(out=b_attn_sb, in_=bat)
                setup_read_names.append(rb.ins.name)
                v_sb = ssb.tile([1, H], f32)
                rv = nc.sync.dma_start(out=v_sb, in_=wv)
                setup_read_names.append(rv.ins.name)
                h_nat = ssb.tile([BL, H], f32)
                rh = nc.sync.dma_start(out=h_nat, in_=hid)
                setup_read_names.append(rh.ins.name)
                ones4 = ssb.tile([1, BL], f32)
                nc.vector.memset(ones4, 1.0)

                wht = [
                    ssb.tile([128, E], f32, tag=f"wht{i}", name=f"wht{i}")
                    for i in range(2)
                ]
                for eh in range(2):
                    for cblk in range(4):  # column blocks of W_attn
                        pt = pps.tile([128, 128], f32, tag="s", bufs=2, name="pt_w")
                        nc.tensor.transpose(
                            pt, w_nat[eh][:, cblk * 128 : (cblk + 1) * 128], ident
                        )
                        if cblk < 2:  # Wh columns
                            nc.scalar.copy(
                                out=wht[cblk][:, eh * 128 : (eh + 1) * 128], in_=pt
                            )
                        else:  # We columns -> fp8 value + residual terms
                            hh = cblk - 2
                            sl = slice(eh * 128, (eh + 1) * 128)
                            nc.scalar.activation(
                                out=wet8a[:, hh, sl], in_=pt, func=AF.Copy,
                                scale=32.0,
                            )
                            nc.vector.scalar_tensor_tensor(
                                out=wet8b[:, hh, sl],
                                in0=pt, scalar=32.0, in1=wet8a[:, hh, sl],
                                op0=mybir.AluOpType.mult,
                                op1=mybir.AluOpType.subtract,
                            )

                ht = [
                    ssb.tile([128, BL], f32, tag=f"ht{i}", name=f"ht{i}")
                    for i in range(2)
                ]
                for hh in range(2):
                    pt = pps.tile([128, 128], f32, tag="s", bufs=2, name="pt_h")
                    nc.tensor.transpose(
                        pt[:, :BL], h_nat[:, hh * 128 : (hh + 1) * 128], ident[:BL, :BL]
                    )
                    nc.scalar.copy(out=ht[hh], in_=pt[:, :BL])

                for eh in range(2):
                    pt = pps.tile([128, 128], f32, tag="s", bufs=2, name="pt_v")
                    nc.tensor.transpose(
                        pt[:, :1], v_sb[:, eh * 128 : (eh + 1) * 128], ident[:1, :1]
                    )
                    nc.scalar.copy(out=vt16[eh], in_=pt[:, :1])

                # qb[eh][e, b] = sum_h WhT[h, e] * hT[h, b] + b_attn[e]
                for eh in range(2):
                    pq = pps.tile([128, 128], f32, tag="s", bufs=2, name="pt_q")
                    for hh in range(2):
                        nc.tensor.matmul(
                            pq[:, :BL],
                            wht[hh][:, eh * 128 : (eh + 1) * 128],
                            ht[hh],
                            start=(hh == 0),
                            stop=False,
                        )
                    nc.tensor.matmul(
                        pq[:, :BL],
                        b_attn_sb[:, eh * 128 : (eh + 1) * 128],
                        ones4,
                        start=False,
                        stop=True,
                    )
                    nc.scalar.copy(out=qb[eh], in_=pq[:, :BL])

            # ---------------- main loop ----------------
            # both context halves share one PSUM bank: half h lives at
            # partition rows {32h, 32h+1} (tile_position col offsets), which
            # frees a bank for a 5th energy buffer.
            pctx_bank = ppc.tile([34, 512], f32, tag="ctx", name="pctx")
            pctx = [pctx_bank[32 * h : 32 * h + 2] for h in range(2)]

            def ctx_group(g):
                for half in range(2):
                    for jl in range(4):
                        n = g * 4 + jl
                        nc.tensor.matmul(
                            pctx[half],
                            u_g[g][:, 2 * half : 2 * half + 2, jl],
                            x_res[g][:, jl, half * 512 : (half + 1) * 512],
                            start=(n == 0),
                            stop=(n == NG * 4 - 1),
                            tile_position=(0, 32 * half),
                        )

            def strip_block(g):
                # exp strips of group g -> u_g[g] (PE transposes + DVE copies)
                for c in range(4):
                    pt = pps.tile([128, 256], bf16, tag="s", bufs=2, name="pt_u")
                    nc.tensor.transpose(
                        pt[:, :97],
                        st_g[g][:, c * 128 : (c + 1) * 128],
                        ident16[:97, :97],
                    )
                    nc.vector.tensor_copy(
                        out=u_g[g][:, :, c],
                        in_=pt.rearrange("p (a r) -> p a r", r=32)[:, :4, 0],
                    )

            def cast_group(g, ihalf=None):
                # x_res[g] (i-slice) -> x8[g] with h-halves interleaved.
                ii = range(4) if ihalf is None else range(2 * ihalf, 2 * ihalf + 2)
                names = []
                for i in ii:
                    src = x_res[g][:, i].rearrange("p (b hh k) -> p b hh k", b=4, hh=2)
                    dst = x8[g].rearrange(
                        "p (i b k hh) -> p i b hh k", i=4, b=4, hh=2
                    )[:, i]
                    cc = nc.vector.tensor_copy(out=dst, in_=src)
                    names.append(cc.ins.name)
                return names

            def tpose_group(g, ihalf=None):
                # fp8-pair xbar transpose of x8[g] (i-slice) into xt8[g].
                # Transport dtype is bf16 (bitcast): the xbar moves 16-bit
                # elements regardless, and the bf16-typed path runs ~2x the
                # element rate of uint16 in practice.
                if ihalf is None:
                    src = x8[g].bitcast(bf16)
                    dst = xt8[g].bitcast(bf16).rearrange("p (grp s) -> p grp s", s=128)
                else:
                    src = x8[g][:, 2048 * ihalf : 2048 * (ihalf + 1)].bitcast(bf16)
                    dst = xt8[g][:, 2048 * ihalf : 2048 * (ihalf + 1)].bitcast(
                        bf16
                    ).rearrange("p (grp s) -> p grp s", s=128)
                return nc.sync.dma_start_transpose(dst, src)

            # fp8 X^T DoubleRow views: rhs(g, b) = [128 p, 2 hh, 4 i, 128 s];
            # the hh pair is fp8-adjacent (stride 1), so the DoubleRow ifmap
            # fetch reads contiguous 16-bit pairs at 2 rows/cycle.
            def rhs_view(g, b):
                v = xt8[g].rearrange(
                    "p (i b s hh) -> p b hh i s", i=4, b=4, hh=2
                )
                return v[:, b]

            for g in range(NG):
                for ih in range(2):
                    cnames = cast_group(g, ih)
                    tp = tpose_group(g, ih)
                    tpose_deps.append((tp.ins.name, cnames))
                    tpose_of_group.setdefault(g, []).append(tp.ins.name)

                strip = pps.tile([97, 512], f32, tag="s", name="strip")

                def energy_block(eh):
                    pe_t = [
                        ppe.tile([128, 512], f32, tag="e", name=f"pe{b}")
                        for b in range(BL)
                    ]
                    for term in range(2):
                        wt = wet8a if term == 0 else wet8b
                        for b in range(BL):
                            nc.tensor.matmul(
                                pe_t[b],
                                wt[:, :, eh * 128 : (eh + 1) * 128],
                                rhs_view(g, b),
                                start=(term == 0),
                                stop=(term == 1),
                                perf_mode=mybir.MatmulPerfMode.DoubleRow,
                            )
                    th_eh = []
                    for b in range(BL):
                        th = thp.tile([128, 512], bf16, tag="th", name="th")
                        nc.scalar.activation(
                            out=th,
                            in_=pe_t[b],
                            func=AF.Tanh,
                            bias=qb[eh][:, b : b + 1],
                            scale=1.0 / 32.0,
                        )
                        th_eh.append(th)
                    return th_eh

                def vdot_block(eh, th_eh):
                    for b in range(BL):
                        nc.tensor.matmul(
                            strip[32 * b : 32 * b + 1, :],
                            vt16[eh],
                            th_eh[b],
                            start=(eh == 0),
                            stop=(eh == 1),
                            tile_position=(0, 32 * b),
                        )

                # The previous group's strip transposes + ctx first: they
                # fill the PE while this group's transpose lands, and the
                # 4-deep PSUM pool chains energy(g) behind tanh(g-1) anyway.
                if g >= 1:
                    strip_block(g - 1)
                    ctx_group(g - 1)
                th0 = energy_block(0)
                vdot_block(0, th0)
                th1 = energy_block(1)
                vdot_block(1, th1)

                nc.scalar.activation(
                    out=st_g[g],
                    in_=strip,
                    func=AF.Exp,
                    accum_out=acc_all[:, g : g + 1],
                )

            strip_block(NG - 1)

            # denominators ship first so their HBM-write receipt hides
            # under the context tail
            accs = wp.tile([97, 1], f32)
            nc.vector.reduce_sum(out=accs, in_=acc_all, axis=mybir.AxisListType.X)
            nc.sync.dma_start(out=den_t.ap(), in_=accs)

            g = NG - 1
            for half in range(2):
                for jl in range(4):
                    n = g * 4 + jl
                    nc.tensor.matmul(
                        pctx[half],
                        u_g[g][:, 2 * half : 2 * half + 2, jl],
                        x_res[g][:, jl, half * 512 : (half + 1) * 512],
                        start=(n == 0),
                        stop=(n == NG * 4 - 1),
                        tile_position=(0, 32 * half),
                    )
                csb = wp.tile([2, 512], f32, tag="csb", name=f"csb{half}")
                nc.scalar.copy(out=csb, in_=pctx[half])
                nc.sync.dma_start(out=ctxu_t.ap()[half], in_=csb)

    pace_deps = {
        read_of_group[g][0]: [tpose_of_group[g - 3][0]] for g in range(3, NG)
    }
    _dma_surgery(nc, mybir, read_names, tpose_deps, setup_read_names,
                 pace_deps=pace_deps, verbose=verbose)
    _split_multiwait(nc, mybir)
    return nc


def kernel(**inputs):
    from concourse.bass_utils import run_bass_kernel_spmd

    hidden = np.asarray(inputs["hidden"], dtype=np.float32)
    enc = np.asarray(inputs["encoder_outputs"], dtype=np.float32)
    w_attn = np.ascontiguousarray(np.asarray(inputs["W_attn"], dtype=np.float32))
    b_attn = np.ascontiguousarray(np.asarray(inputs["b_attn"], dtype=np.float32))
    w_v = np.ascontiguousarray(np.asarray(inputs["W_v"], dtype=np.float32))

    if "nc" not in _CACHE:
        _CACHE["nc"] = _build()
    nc = _CACHE["nc"]

    in_maps = []
    for c in range(NCORES):
        sl = slice(c * BL, (c + 1) * BL)
        in_maps.append(
            {
                "hidden": np.ascontiguousarray(hidden[sl]),
                "enc": np.ascontiguousarray(enc[:, sl, :]),
                "w_attn": w_attn,
                "b_attn": b_attn,
                "w_v": w_v,
            }
        )

    trace = bool(_CACHE.get("trace", False))
    res = run_bass_kernel_spmd(nc, in_maps, core_ids=list(range(NCORES)), trace=trace)
    _CACHE["last_results"] = res

    out = np.empty((1, B, H), dtype=np.float32)
    for c in range(NCORES):
        ctxu = res.results[c]["ctxu"]  # [2, 2, 512]
        den = res.results[c]["den"]    # [97, 1]
        for b in range(BL):
            half, row = b // 2, b % 2
            vals = ctxu[half, row, row * 256 : row * 256 + 256]
            out[0, c * BL + b] = vals / den[32 * b, 0]
    return out
